# revision 5
# baseline (speedup 1.0000x reference)
"""BiLSTM-CRF NLL kernel for Trainium2 (8 NeuronCores, SPMD).

Sharding: 8 cores = 2 directions x 4 batch-quarters (BC=16 sequences per
core); fwd/bwd pairs exchange partial emissions with an AllGather, CRF
runs on the fwd cores. Optimized for instruction count (the workload is
small-op dominated, so per-instruction dispatch/sync overhead rules):

- Embedding gather + transpose on HOST; x ships as fp8e3 [EMB, S*BC]
  (~2MB/core), upcast + spread to 32-col step slots on device.
- BATCH-MAJOR recurrence: gates [16, 1024] via matmuls with the weight
  matrices as the MOVING operand; the input projection + bias is batched
  3 steps per stationary (out rows at 32-aligned partition slots), so a
  step costs ~6 chain matmuls + 2 tiny PE transposes of h (back to
  h-major lhsT layout) + 7 elementwise ops — ~23 engine instructions
  versus 39+ for the gate-major formulation.
- Chunk-parallel CRF: 8 time-chunk operators (17x17 per sequence, exp
  space, renormalized every 8 steps) evolve together, each local step
  being 5 wide matmuls + 1 broadcast-multiply; the final 8 tiny matvecs
  per sequence run on host from the downloaded operators. Serial depth
  64 instead of 511.
- Post-schedule passes: duplicate-Ldweights removal, transitive sem-wait
  pruning, and same-engine-order wait dropping kill ~2k Drain/Ldweights
  instructions that the stock Tile lowering emits.
"""

import numpy as np
import ml_dtypes

import bass_rust
import concourse.bass as bass
import concourse.mybir as mybir
import concourse.tile as tile
from concourse.bass_utils import run_bass_kernel_spmd
from concourse.vector_clock import ScopedClock


def _split_drain_and_barrier(self, tick_clock, wait_clock):
    """TileContext tail-drain patch: the walrus in this container rejects >1
    sync wait on a Drain (CoreV3 CTRL_NO_STRUCT), so split the final
    global-clock waits across one Drain per semaphore."""
    nc = self.nc
    drain_inst = nc.sync.drain()
    wait_clock.add_sem_waits(
        drain_inst.ins, ScopedClock({None: tick_clock.global_clock}))
    si = drain_inst.ins.sync_info
    if si is not None and si.on_wait and len(si.on_wait) > 1:
        waits = list(si.on_wait)
        drain_inst.ins.sync_info = bass_rust.SyncInfo(
            on_wait=[waits[0]], on_update=list(si.on_update))
        for w in waits[1:]:
            extra = nc.sync.drain()
            extra.ins.sync_info = bass_rust.SyncInfo(on_wait=[w], on_update=[])
    nc.all_engine_barrier()
    assert self.sems is not None
    popped = nc._tile_sem_poison_stack.pop()
    assert popped is self._sem_poison
    nc.clear_and_free_semaphores(list(self.sems.allocated().values()))
    nc.all_engine_barrier()


tile.TileContext._drain_and_barrier = _split_drain_and_barrier

_orig_lower_ordered_insts = tile.TileContext._lower_ordered_insts


def _dedup_ldweights(postordered_blocks):
    """Remove InstLdweights whose stationary AP is identical to the one
    already loaded (PE keeps the stationary across matmuls with
    ldweights=False). Waits/updates of a removed load are merged into the
    next PE instruction."""
    removed = 0
    for bb_name, insts in postordered_blocks.items():
        last_key = None
        pend_wait, pend_upd = [], []
        out = []
        for inst in insts:
            if isinstance(inst, mybir.InstLdweights):
                key = str(inst.ins[0])
                if key == last_key:
                    si = inst.sync_info
                    if si is not None:
                        pend_wait.extend(si.on_wait or [])
                        pend_upd.extend(si.on_update or [])
                    removed += 1
                    continue
                last_key = key
            elif (pend_wait or pend_upd) and \
                    getattr(inst, "engine", None) == mybir.EngineType.PE:
                si = inst.sync_info
                ow = list(si.on_wait) if si is not None else []
                ou = list(si.on_update) if si is not None else []
                inst.sync_info = bass_rust.SyncInfo(
                    on_wait=pend_wait + ow, on_update=pend_upd + ou)
                pend_wait, pend_upd = [], []
            out.append(inst)
        assert not pend_wait and not pend_upd
        insts[:] = out
    return removed


def _merge_same_sem_waits(waits):
    """Collapse sem-ge waits on the same semaphore to the max value."""
    merged = {}
    out = []
    for w in waits:
        if getattr(w, "wait_mode", None) == "sem-ge-imm" and \
                getattr(w, "wait_reg", None) is None:
            key = (w.sync_type, w.id)
            if key in merged:
                prev = merged[key]
                if w.wait_value > prev.wait_value:
                    out[out.index(prev)] = w
                    merged[key] = w
                continue
            merged[key] = w
        out.append(w)
    return out


# engines with strictly serial in-order execution (full drain between
# ops). Pool (GPSIMD) is excluded: its 8 Q7 cores run concurrently. SP is
# excluded: its dma_starts complete asynchronously, so SP program order
# does not imply its semaphore values.
_SELF_WAIT_ENGINES = (mybir.EngineType.DVE, mybir.EngineType.Activation)


def _transitive_prune_waits(postordered_blocks):
    """Prune waits implied transitively through other waits: if inst X
    waits on sem A at value a, and A's producer itself waited on sem B >=
    b, then an X-wait on B >= b' with b' <= b is redundant. Tracks, per
    single-engine-owned semaphore value, a snapshot of the guaranteed
    vector clock at the producing instruction."""
    from collections import defaultdict
    pruned = 0
    for bb_name, insts in postordered_blocks.items():
        upd_engines = defaultdict(set)
        for inst in insts:
            si = getattr(inst, "sync_info", None)
            if si is not None:
                for u in (si.on_update or []):
                    upd_engines[u.id].add(getattr(inst, "engine", None))
        tracked = {sid for sid, es in upd_engines.items() if len(es) == 1}
        cur = defaultdict(int)        # sem -> current cumulative value
        snap = defaultdict(dict)      # sem -> value -> know dict
        know = defaultdict(dict)      # engine -> sem -> guaranteed value
        for inst in insts:
            si = getattr(inst, "sync_info", None)
            eng = getattr(inst, "engine", None)
            ke = know[eng]
            if si is not None and si.on_wait:
                waits = list(si.on_wait)
                snaps = []
                for w in waits:
                    if (w.sync_type == "semaphore" and w.id in tracked
                            and w.wait_mode == "sem-ge-imm"
                            and getattr(w, "wait_reg", None) is None):
                        snaps.append(snap[w.id].get(w.wait_value, {w.id: w.wait_value}))
                    else:
                        snaps.append(None)
                if len(waits) > 1:
                    kept = []
                    for i, w in enumerate(waits):
                        if (w.sync_type == "semaphore"
                                and w.wait_mode == "sem-ge-imm"
                                and getattr(w, "wait_reg", None) is None):
                            implied = ke.get(w.id, 0) >= w.wait_value
                            if not implied:
                                for j, s in enumerate(snaps):
                                    if j != i and s is not None and \
                                            s.get(w.id, 0) >= w.wait_value:
                                        implied = True
                                        break
                            if implied:
                                pruned += 1
                                continue
                        kept.append(w)
                    if len(kept) != len(waits) and kept:
                        inst.sync_info = bass_rust.SyncInfo(
                            on_wait=kept, on_update=list(si.on_update))
                # merge all waits' knowledge (conditions hold either way)
                for w, s in zip(waits, snaps):
                    if s is not None:
                        for sid, v in s.items():
                            if ke.get(sid, 0) < v:
                                ke[sid] = v
                    if w.sync_type == "semaphore" and \
                            w.wait_mode == "sem-ge-imm" and \
                            getattr(w, "wait_reg", None) is None:
                        if ke.get(w.id, 0) < w.wait_value:
                            ke[w.id] = w.wait_value
            si = getattr(inst, "sync_info", None)
            if si is not None:
                for u in (si.on_update or []):
                    if u.id in tracked and u.update_mode == "sem-inc" \
                            and getattr(u, "update_reg", None) is None:
                        cur[u.id] += u.update_value or 0
                        s = dict(ke)
                        s[u.id] = max(s.get(u.id, 0), cur[u.id])
                        snap[u.id][cur[u.id]] = s
                        if DROP_SELF_WAITS and \
                                ke.get(u.id, 0) < cur[u.id] and \
                                eng in _SELF_WAIT_ENGINES and \
                                upd_engines[u.id] == {eng}:
                            ke[u.id] = cur[u.id]
    return pruned

# CoreSim's race detector does not model same-engine serial ordering, so
# sim-based checkers disable this pass; hardware runs keep it on.
DROP_SELF_WAITS = True


def _drop_self_engine_waits(postordered_blocks):
    """Drop waits that are implied by same-engine program order: DVE/ACT/
    SP/Pool execute serially (full drain between ops), so a wait on the
    engine's own monotonic semaphore for a value already reached by
    preceding same-engine increments in this block is a no-op."""
    from collections import defaultdict
    dropped = 0
    for bb_name, insts in postordered_blocks.items():
        upd_engines = defaultdict(set)
        for inst in insts:
            si = getattr(inst, "sync_info", None)
            if si is not None:
                for u in (si.on_update or []):
                    upd_engines[u.id].add(getattr(inst, "engine", None))
        safe = {sid for sid, es in upd_engines.items()
                if len(es) == 1 and next(iter(es)) in _SELF_WAIT_ENGINES}
        cum = defaultdict(int)
        for inst in insts:
            si = getattr(inst, "sync_info", None)
            eng = getattr(inst, "engine", None)
            if si is not None and si.on_wait and len(si.on_wait) > 1:
                kept = []
                for w in si.on_wait:
                    if (w.sync_type == "semaphore" and w.id in safe
                            and next(iter(upd_engines[w.id])) == eng
                            and w.wait_mode == "sem-ge-imm"
                            and getattr(w, "wait_reg", None) is None
                            and cum[w.id] >= w.wait_value):
                        dropped += 1
                        continue
                    kept.append(w)
                if len(kept) != len(si.on_wait):
                    inst.sync_info = bass_rust.SyncInfo(
                        on_wait=kept, on_update=list(si.on_update))
            si = getattr(inst, "sync_info", None)
            if si is not None:
                for u in (si.on_update or []):
                    if u.id in safe and u.update_mode == "sem-inc" \
                            and getattr(u, "update_reg", None) is None:
                        cum[u.id] += u.update_value or 0
    return dropped


def _split_multi_waits(self, postordered_blocks):
    """Same walrus limitation for scheduled instructions: move excess sync
    waits onto same-engine Drain instructions inserted just before."""
    _dedup_ldweights(postordered_blocks)
    _transitive_prune_waits(postordered_blocks)
    if DROP_SELF_WAITS:
        _drop_self_engine_waits(postordered_blocks)
    for bb_name, insts in postordered_blocks.items():
        out = []
        for inst in insts:
            si = getattr(inst, "sync_info", None)
            if si is not None and si.on_wait and len(si.on_wait) > 1:
                merged = _merge_same_sem_waits(list(si.on_wait))
                if len(merged) != len(si.on_wait):
                    inst.sync_info = bass_rust.SyncInfo(
                        on_wait=merged, on_update=list(si.on_update))
                    si = inst.sync_info
            if si is not None and si.on_wait and len(si.on_wait) > 1:
                waits = list(si.on_wait)
                for k, w in enumerate(waits[1:]):
                    d = mybir.InstDrain(
                        name=f"{inst.name}_ws{k}", engine=inst.engine,
                        ins=[], outs=[],
                        sync_info=bass_rust.SyncInfo(on_wait=[w],
                                                     on_update=[]))
                    out.append(d)
                inst.sync_info = bass_rust.SyncInfo(
                    on_wait=[waits[0]], on_update=list(si.on_update))
            out.append(inst)
        insts[:] = out
    return _orig_lower_ordered_insts(self, postordered_blocks)


tile.TileContext._lower_ordered_insts = _split_multi_waits

F32 = mybir.dt.float32
BF16 = mybir.dt.bfloat16
I32 = mybir.dt.int32
AF = mybir.ActivationFunctionType
ALU = mybir.AluOpType

VOCAB, EMB, HID, NLAB = 20000, 256, 512, 17
H = HID // 2          # 256 per direction
GATES = 4 * H         # 1024
B_FULL, S_FULL = 64, 512
KH = H // 128         # 2
KE = EMB // 128       # 2
BANK = 512            # fp32 elems per PSUM bank


def build_nc(S=S_FULL, BC=16, CHUNK=64, RENORM=8, n_cores=8,
             use_collective=True, phases=4):
    """Build the SPMD Bass program (identical on all cores)."""
    assert S % CHUNK == 0
    NCH = S // CHUNK              # emission chunks
    TOK = S * BC                  # tokens per core
    TPC = CHUNK * BC              # tokens per emission chunk
    assert TPC <= 2 * BANK

    nc = bass.Bass("TRN2", target_bir_lowering=False, num_devices=n_cores)

    # ---------------- DRAM I/O ----------------
    xT_d = nc.dram_tensor("xT", [EMB, TOK], mybir.dt.float8e3,
                          kind="ExternalInput")
    wstat_d = nc.dram_tensor("w_stat", [H, GATES], BF16, kind="ExternalInput")
    win_d = nc.dram_tensor("w_in", [EMB, GATES], BF16, kind="ExternalInput")
    brow_d = nc.dram_tensor("bias_row", [1, GATES], BF16, kind="ExternalInput")
    id_d = nc.dram_tensor("ident", [BC, BC], BF16, kind="ExternalInput")
    wo_d = nc.dram_tensor("wo_stat", [H, NLAB], BF16, kind="ExternalInput")
    bo_d = nc.dram_tensor("bo_row", [1, NLAB], BF16, kind="ExternalInput")
    expT_d = nc.dram_tensor("expT", [NLAB, NLAB], F32, kind="ExternalInput")
    expS_d = nc.dram_tensor("expStart", [NLAB, 1], F32, kind="ExternalInput")
    oh_d = nc.dram_tensor("onehot", [NLAB, TOK], BF16, kind="ExternalInput")
    CCH = 8                       # CRF time chunks (parallel operators)
    LCH = S // CCH                # local steps per chunk
    CB = CCH * BC                 # operator blocks
    W = CB * NLAB                 # state columns
    crfinit_d = nc.dram_tensor("crf_init", [NLAB, W], mybir.dt.float8e3,
                               kind="ExternalInput")
    out_emit_d = nc.dram_tensor("out_emit", [NLAB, BC], F32,
                                kind="ExternalOutput")
    out_R_d = nc.dram_tensor("out_R", [NLAB, W], F32,
                             kind="ExternalOutput")
    out_logs_d = nc.dram_tensor("out_logs", [1, CB], F32,
                                kind="ExternalOutput")
    if use_collective:
        cc_in_d = nc.dram_tensor("cc_in", [NLAB, TOK], F32, kind="Internal")
        cc_out_d = nc.dram_tensor("cc_out", [2, NLAB, TOK], F32,
                                  kind="Internal")
    else:
        emf_in_d = nc.dram_tensor("dbg_em_f", [NLAB, TOK], F32,
                                  kind="ExternalInput")
        emb_in_d = nc.dram_tensor("dbg_em_b", [NLAB, TOK], F32,
                                  kind="ExternalInput")
        dbg_out_d = nc.dram_tensor("dbg_em_out", [NLAB, TOK], F32,
                                   kind="ExternalOutput")

    groups = [[i, i + n_cores // 2] for i in range(n_cores // 2)]

    with tile.TileContext(nc) as tc:
        with tc.tile_pool(name="consts", bufs=1) as consts, \
             tc.tile_pool(name="state", bufs=1) as state:
            # ---- constants needed beyond phase 1 ----
            ones_row = consts.tile([1, BANK], BF16, tag="ones_row")
            nc.vector.memset(ones_row[:], 1.0)
            wo_sb = consts.tile([128, KH * NLAB], BF16, tag="wo")
            for k in range(KH):
                nc.sync.dma_start(wo_sb[:, k * NLAB:(k + 1) * NLAB],
                                  wo_d[128 * k:128 * (k + 1), :])
            bo_sb = consts.tile([1, NLAB], BF16, tag="bo")
            nc.sync.dma_start(bo_sb[:], bo_d[:])

            # ---- LSTM state ----
            hs_all = state.tile([128, S + 1, KH, BC], BF16, tag="hs")
            nc.vector.memset(hs_all[:, 0], 0.0)
            c_st = state.tile([BC, H], F32, tag="c")
            nc.vector.memset(c_st[:], 0.0)

            # =============== phase 1: LSTM recurrence (batch-major) ======
            with tc.tile_pool(name="lstmc", bufs=1) as lstmc, \
                 tc.tile_pool(name="gpsum", bufs=1, space="PSUM") as psum, \
                 tc.tile_pool(name="step", bufs=3) as step_pool:
                # LSTM-only constants; pool closes after phase 1 to free
                # SBUF for the emission tiles.
                wk = []
                for k in range(KH):
                    t = lstmc.tile([128, GATES], BF16, name=f"wk{k}",
                                   tag=f"wk{k}")
                    nc.sync.dma_start(t[:], wstat_d[128 * k:128 * (k + 1), :])
                    wk.append(t)
                wi = []
                for k in range(KE):
                    t = lstmc.tile([128, GATES], BF16, name=f"wi{k}",
                                   tag=f"wi{k}")
                    nc.sync.dma_start(t[:], win_d[128 * k:128 * (k + 1), :])
                    wi.append(t)
                brow = lstmc.tile([1, GATES], BF16, tag="brow")
                nc.sync.dma_start(brow[:], brow_d[:])
                ident = lstmc.tile([BC, BC], BF16, tag="ident")
                nc.sync.dma_start(ident[:], id_d[:])
                # xT arrives packed fp8 [EMB, S*BC]; upcast to bf16 and
                # spread to 32-col step slots (zero padding between
                # batches) so batched 3-step stationaries hit 32-aligned
                # row slots.
                xT = []
                for k in range(KE):
                    t8 = lstmc.tile([128, TOK], mybir.dt.float8e3,
                                    name=f"xT8_{k}", tag=f"xT8_{k}")
                    nc.sync.dma_start(t8[:], xT_d[128 * k:128 * (k + 1), :])
                    t = lstmc.tile([128, S * 32], BF16, name=f"xT{k}",
                                   tag=f"xT{k}")
                    nc.vector.memset(t[:], 0.0)
                    nc.vector.tensor_copy(
                        t[:].rearrange("p (s v) -> p s v", s=S,
                                       v=32)[:, :, 0:BC],
                        t8[:].rearrange("p (s b) -> p s b", s=S, b=BC))
                    xT.append(t)
                gp = psum.tile([128, 2, GATES], F32, tag="gp")
                tp = psum.tile([128, KH * BC], F32, tag="tp")

                HB = GATES // 2   # psum-bank-sized half (512 f32)
                GRP = 3           # steps per psum group (32-row slots;
                                  # hw limits AP base partition to 0/32/64)
                starts = list(range(0, S, GRP))
                for g, s0 in enumerate(starts):
                    n = min(GRP, S - s0)
                    par = g % 2
                    # off-chain: batched input projection + bias for the
                    # 4 steps of this group. xT is host-padded to 32 cols
                    # per step so the stationary [128,128] lands each
                    # step's 16 batch rows at a 32-aligned partition slot.
                    for k in range(KE):
                        xs = xT[k][:, s0 * 32:(s0 + n) * 32]
                        for hf in range(2):
                            nc.tensor.matmul(
                                gp[0:32 * n, par, hf * HB:(hf + 1) * HB],
                                xs, wi[k][:, hf * HB:(hf + 1) * HB],
                                start=(k == 0), stop=False,
                                skip_group_check=True)
                    for hf in range(2):
                        nc.tensor.matmul(
                            gp[0:32 * n, par, hf * HB:(hf + 1) * HB],
                            ones_row[:, :32 * n],
                            brow[:, hf * HB:(hf + 1) * HB],
                            start=False, stop=False, skip_group_check=True)
                    for m in range(n):
                        s = s0 + m
                        ro = 32 * m
                        # chain: recurrent part into this step's row slot
                        for k in range(KH):
                            for hf in range(2):
                                nc.tensor.matmul(
                                    gp[ro:ro + BC, par,
                                       hf * HB:(hf + 1) * HB],
                                    hs_all[:, s, k, :],
                                    wk[k][:, hf * HB:(hf + 1) * HB],
                                    start=False, stop=(k == KH - 1),
                                    skip_group_check=True)
                        # elementwise, batch-major [16, *]; gate order
                        # i,f,o,g; reads shift partitions 32m -> 0
                        T = step_pool.tile([BC, GATES], F32, tag="T")
                        nc.scalar.activation(T[:, 0:3 * H],
                                             gp[ro:ro + BC, par, 0:3 * H],
                                             AF.Sigmoid)
                        nc.scalar.activation(T[:, 3 * H:],
                                             gp[ro:ro + BC, par, 3 * H:],
                                             AF.Tanh)
                        Q = step_pool.tile([BC, H], F32, tag="Q")
                        R = step_pool.tile([BC, H], F32, tag="R")
                        # R first: it only needs the sigmoid pass, so the
                        # in-order DVE can start it while tanh(g) runs
                        nc.vector.tensor_tensor(R[:], T[:, H:2 * H],
                                                c_st[:], op=ALU.mult)
                        nc.vector.tensor_tensor(Q[:], T[:, 0:H],
                                                T[:, 3 * H:], op=ALU.mult)
                        nc.vector.tensor_tensor(c_st[:], Q[:], R[:],
                                                op=ALU.add)
                        tc_t = step_pool.tile([BC, H], F32, tag="tc")
                        nc.scalar.activation(tc_t[:], c_st[:], AF.Tanh)
                        hbm = step_pool.tile([BC, H], BF16, tag="hbm")
                        nc.vector.tensor_tensor(hbm[:], T[:, 2 * H:3 * H],
                                                tc_t[:], op=ALU.mult)
                        # transpose h back to h-major for the next lhsT
                        for k in range(KH):
                            nc.tensor.matmul(tp[:, k * BC:(k + 1) * BC],
                                             hbm[:, k * 128:(k + 1) * 128],
                                             ident[:], start=True,
                                             stop=True)
                        nc.scalar.copy(hs_all[:, s + 1], tp[:])

            # =============== phase 2: partial emissions ===============
            if phases < 2:
                return nc
            with tc.tile_pool(name="emis", bufs=1) as emis:
                em_my = emis.tile([NLAB, TOK], F32, tag="em_my")
                with tc.tile_pool(name="empsum", bufs=2,
                                  space="PSUM") as em_ps_p:
                    HCK = CHUNK // 2
                    for ch in range(NCH):
                        ep = em_ps_p.tile([NLAB, TPC], F32, tag="ep")
                        for hf in range(2):
                            es = ep[:, hf * BANK:(hf + 1) * BANK]
                            nc.tensor.matmul(es, bo_sb[:],
                                             ones_row[:, :BANK],
                                             start=True, stop=False)
                            a = ch * CHUNK + hf * HCK + 1
                            for kc in range(KH):
                                rhs = hs_all[:, a:a + HCK, kc, :]
                                nc.tensor.matmul(
                                    es,
                                    wo_sb[:, kc * NLAB:(kc + 1) * NLAB],
                                    rhs, start=False, stop=(kc == KH - 1))
                        nc.scalar.copy(em_my[:, ch * TPC:(ch + 1) * TPC],
                                       ep[:])

                # =============== phase 3: exchange + CRF inputs ========
                if phases < 3:
                    return nc
                if not use_collective:
                    nc.sync.dma_start(dbg_out_d[:], em_my[:])
                if use_collective:
                    nc.sync.dma_start(cc_in_d[:], em_my[:])
                    nc.gpsimd.collective_compute(
                        "AllGather", ALU.bypass, replica_groups=groups,
                        ins=[cc_in_d[:]], outs=[cc_out_d[:]])
                em_f = emis.tile([NLAB, TOK], F32, tag="em_f")
                em_b = emis.tile([NLAB, TOK], F32, tag="em_b")
                if use_collective:
                    nc.sync.dma_start(em_f[:], cc_out_d[0])
                    nc.sync.dma_start(em_b[:], cc_out_d[1])
                else:
                    nc.sync.dma_start(em_f[:], emf_in_d[:])
                    nc.sync.dma_start(em_b[:], emb_in_d[:])
                em_b_rev = em_b[:].rearrange("p (s b) -> p s b",
                                             s=S, b=BC)[:, ::-1, :]
                nc.vector.tensor_tensor(em_f[:], em_f[:], em_b_rev,
                                        op=ALU.add)
                eem = emis.tile([NLAB, TOK], F32, tag="eem")
                nc.scalar.activation(eem[:], em_f[:], AF.Exp)

                # gold-label emission sums
                oh_sb = emis.tile([NLAB, TOK], BF16, tag="oh")
                nc.sync.dma_start(oh_sb[:], oh_d[:])
                nc.vector.tensor_tensor(em_b[:], em_f[:], oh_sb[:],
                                        op=ALU.mult)
                emit_bt = emis.tile([NLAB, BC], F32, tag="emit_bt")
                nc.vector.tensor_reduce(
                    emit_bt[:],
                    em_b[:].rearrange("p (s b) -> p b s", s=S, b=BC),
                    axis=mybir.AxisListType.X, op=ALU.add)
                nc.sync.dma_start(out_emit_d[:], emit_bt[:])

                # =============== phase 4: CRF forward scan =============
                if phases < 4:
                    return nc
                # Chunk-parallel CRF: evolve 8 time-chunk operators (17x17
                # per sequence, exp space) simultaneously; each local step
                # is 5 wide matmuls + 1 broadcast-multiply instead of a
                # 511-long matmul->multiply serial chain. The tiny final
                # operator chain (8 matvecs per sequence) runs on host.
                with tc.tile_pool(name="crfc", bufs=1) as crf_c, \
                     tc.tile_pool(name="crfp", bufs=2) as crf_p, \
                     tc.tile_pool(name="crfps", bufs=1,
                                  space="PSUM") as crf_ps:
                    expT_sb = crf_c.tile([NLAB, NLAB], F32, tag="expT")
                    nc.sync.dma_start(expT_sb[:], expT_d[:])
                    expS_sb = crf_c.tile([NLAB, 1], F32, tag="expS")
                    nc.sync.dma_start(expS_sb[:], expS_d[:])
                    ones17 = crf_c.tile([NLAB, 1], F32, tag="ones17")
                    nc.vector.memset(ones17[:], 1.0)
                    ones117 = crf_c.tile([1, NLAB], F32, tag="ones117")
                    nc.vector.memset(ones117[:], 1.0)
                    logs = crf_c.tile([1, CB], F32, tag="logs")
                    nc.vector.memset(logs[:], 0.0)
                    Rst = crf_c.tile([NLAB, W], F32, tag="Rst")
                    ini8 = crf_c.tile([NLAB, W], mybir.dt.float8e3,
                                      tag="ini8")
                    nc.sync.dma_start(ini8[:], crfinit_d[:])
                    nc.vector.tensor_copy(Rst[:], ini8[:])
                    qt = crf_ps.tile([NLAB, W], F32, tag="qt")

                    eemv = eem[:].rearrange("p (c t b) -> p c t b",
                                            c=CCH, t=LCH, b=BC)
                    Rv = Rst[:].rearrange("p (c b j) -> p c b j",
                                          c=CCH, b=BC, j=NLAB)

                    def regions(lo, hi):
                        edges = [lo] + [e for e in range(BANK, hi, BANK)
                                        if e > lo] + [hi]
                        return zip(edges[:-1], edges[1:])

                    BW = BC * NLAB   # columns per chunk block (272)
                    for t in range(LCH):
                        lo = BW if t == 0 else 0
                        for (ra, rb) in regions(lo, W):
                            nc.tensor.matmul(qt[:, ra:rb], expT_sb[:],
                                             Rst[:, ra:rb],
                                             start=True, stop=True)
                        if t == 0:
                            # chunk 0 starts as the plain alpha vector
                            nc.vector.tensor_scalar_mul(
                                Rv[:, 0, :, 0], eem[:, 0:BC], expS_sb[:])
                            em_bc = eemv[:, 1:, t, :].broadcast_to(
                                (NLAB, CCH - 1, BC, NLAB))
                            nc.vector.tensor_tensor(
                                Rv[:, 1:], qt[:, BW:].rearrange(
                                    "p (c b j) -> p c b j", c=CCH - 1,
                                    b=BC, j=NLAB),
                                em_bc, op=ALU.mult)
                        else:
                            em_bc = eemv[:, :, t, :].broadcast_to(
                                (NLAB, CCH, BC, NLAB))
                            nc.vector.tensor_tensor(
                                Rv[:], qt[:].rearrange(
                                    "p (c b j) -> p c b j", c=CCH,
                                    b=BC, j=NLAB),
                                em_bc, op=ALU.mult)
                        if t % RENORM == RENORM - 1:
                            Rj = crf_p.tile([NLAB, CB], F32, tag="Rj")
                            nc.vector.tensor_reduce(
                                Rj[:], Rv[:],
                                axis=mybir.AxisListType.X, op=ALU.add)
                            cs = crf_ps.tile([1, CB], F32, tag="cs")
                            nc.tensor.matmul(cs[:], ones17[:], Rj[:],
                                             start=True, stop=True)
                            sinv = crf_p.tile([1, CB], F32, tag="sinv")
                            nc.vector.reciprocal(sinv[:], cs[:])
                            bc17 = crf_ps.tile([NLAB, CB], F32,
                                               tag="bc17")
                            nc.tensor.matmul(bc17[:], ones117[:],
                                             sinv[:], start=True,
                                             stop=True)
                            sc_bc = bc17[:].rearrange(
                                "p (c b) -> p c b", c=CCH,
                                b=BC).broadcast_to(
                                (NLAB, CCH, BC, NLAB))
                            nc.vector.tensor_tensor(Rv[:], Rv[:], sc_bc,
                                                    op=ALU.mult)
                            lg = crf_p.tile([1, CB], F32, tag="lg")
                            nc.scalar.activation(lg[:], cs[:], AF.Ln)
                            nc.vector.tensor_tensor(logs[:], logs[:],
                                                    lg[:], op=ALU.add)
                    nc.sync.dma_start(out_R_d[:], Rst[:])
                    nc.sync.dma_start(out_logs_d[:], logs[:])

    return nc


# ====================== host side ======================

def _perm_gates(w, order=(0, 1, 3, 2)):
    """reorder gate blocks [i,f,g,o] -> [i,f,o,g] along axis 0"""
    blocks = np.split(np.asarray(w), 4, axis=0)
    return np.concatenate([blocks[i] for i in order], axis=0)


def _bf(x):
    return np.ascontiguousarray(
        np.asarray(x, dtype=np.float32)).astype(ml_dtypes.bfloat16)


def make_in_maps(inputs, S=S_FULL, BC=16, n_cores=8, use_collective=True,
                 dbg_em=None):
    chars = np.asarray(inputs["chars"], dtype=np.int64)
    labels = np.asarray(inputs["labels"], dtype=np.int64)
    npair = n_cores // 2
    emb_f = np.asarray(inputs["emb"], np.float32)
    ident = np.eye(BC, dtype=np.float32)

    in_maps = []
    for core in range(n_cores):
        is_bwd = core >= npair
        q = core % npair
        ch_q = chars[q * BC:(q + 1) * BC, :S]          # [BC, S]
        lb_q = labels[q * BC:(q + 1) * BC, :S]
        d = "b" if is_bwd else "f"
        w_ih = _perm_gates(inputs[f"w_ih_{d}"])
        w_hh = _perm_gates(inputs[f"w_hh_{d}"])
        bias = _perm_gates(np.asarray(inputs[f"b_ih_{d}"]) +
                           np.asarray(inputs[f"b_hh_{d}"]))
        ch_dev = ch_q[:, ::-1] if is_bwd else ch_q     # device step order
        # xT [EMB, S*BC], token col = s*BC + b
        x = emb_f[ch_dev]                               # [BC, S, EMB]
        xT = np.ascontiguousarray(
            x.transpose(2, 1, 0).reshape(EMB, S * BC))
        w_out = np.asarray(inputs["w_out"], np.float32)
        wo_half = w_out[:, H:] if is_bwd else w_out[:, :H]
        bo = np.zeros(NLAB, np.float32) if is_bwd \
            else np.asarray(inputs["b_out"], np.float32)
        onehot = (lb_q.T.reshape(1, -1) ==
                  np.arange(NLAB)[:, None]).astype(np.float32)
        m = {
            "xT": xT.astype(ml_dtypes.float8_e3m4),
            "w_stat": _bf(w_hh.T),
            "w_in": _bf(w_ih.T),
            "bias_row": _bf(bias.reshape(1, -1)),
            "ident": ident.astype(ml_dtypes.bfloat16),
            "wo_stat": _bf(wo_half.T),
            "bo_row": _bf(bo.reshape(1, -1)),
            "expT": np.ascontiguousarray(
                np.exp(np.asarray(inputs["trans"], np.float32))),
            "expStart": np.exp(np.asarray(
                inputs["start_trans"], np.float32)).reshape(-1, 1),
            "crf_init": _crf_init(S, BC),
            "onehot": _bf(onehot),
        }
        if not use_collective:
            m["dbg_em_f"] = np.asarray(dbg_em[q][0], np.float32)
            m["dbg_em_b"] = np.asarray(dbg_em[q][1], np.float32)
        in_maps.append(m)
    return in_maps


def _crf_init(S, BC, CCH=8):
    """Initial chunk operators: identity blocks for chunks 1.., zeros for
    chunk 0 (which starts from the alpha vector on device)."""
    init = np.zeros((NLAB, CCH, BC, NLAB), np.float32)
    init[:, 1:, :, :] = np.eye(NLAB, dtype=np.float32)[:, None, None, :]
    return np.ascontiguousarray(
        init.reshape(NLAB, CCH * BC * NLAB)).astype(ml_dtypes.float8_e3m4)


def combine_logz(r, end_trans, S=S_FULL, BC=16, CCH=8):
    """Host tail of the chunk-parallel CRF: chain the 8 chunk operators
    per sequence and apply end transitions. Returns [BC] logZ values."""
    R = np.asarray(r["out_R"], np.float64).reshape(NLAB, CCH, BC, NLAB)
    logs = np.asarray(r["out_logs"], np.float64).reshape(CCH, BC)
    eT = np.exp(np.asarray(end_trans, np.float64))
    out = np.empty(BC)
    for b in range(BC):
        P = R[:, 0, b, 0]
        for c in range(1, CCH):
            P = R[:, c, b, :] @ P
        out[b] = np.log(eT @ P) + logs[:, b].sum()
    return out


def static_score(inputs, S=S_FULL):
    """label-only part of the numerator (host, from inputs only)"""
    labels = np.asarray(inputs["labels"], dtype=np.int64)[:, :S]
    st = np.asarray(inputs["start_trans"], np.float64)
    et = np.asarray(inputs["end_trans"], np.float64)
    tr = np.asarray(inputs["trans"], np.float64)
    sc = st[labels[:, 0]] + et[labels[:, -1]]
    sc = sc + tr[labels[:, :-1], labels[:, 1:]].sum(axis=1)
    return float(sc.sum())


def reduce_outputs(results, inputs, n_cores=8, S=S_FULL):
    total = 0.0
    for q in range(n_cores // 2):
        r = results[q]
        total += float(combine_logz(r, inputs["end_trans"], S=S).sum())
        total -= float(np.asarray(r["out_emit"], np.float64).sum())
    total -= static_score(inputs, S=S)
    return np.float32(total)


def kernel(**inputs) -> np.ndarray:
    S, BC, n_cores = S_FULL, 16, 8
    nc = build_nc(S=S, BC=BC, n_cores=n_cores)
    in_maps = make_in_maps(inputs, S=S, BC=BC, n_cores=n_cores)
    res = run_bass_kernel_spmd(nc, in_maps, core_ids=list(range(n_cores)))
    return reduce_outputs(res.results, inputs, n_cores=n_cores, S=S)


# revision 7
# speedup vs baseline: 1.0238x; 1.0238x over previous
"""BiLSTM-CRF NLL kernel for Trainium2 (8 NeuronCores, SPMD).

Sharding: 8 cores = 2 directions x 4 batch-quarters (BC=16 sequences per
core); fwd/bwd pairs exchange partial emissions with an AllGather, CRF
runs on the fwd cores. Optimized for instruction count (the workload is
small-op dominated, so per-instruction dispatch/sync overhead rules):

- Embedding gather + transpose on HOST; x ships as fp8e3 [EMB, S*BC]
  (~2MB/core), upcast + spread to 32-col step slots on device.
- BATCH-MAJOR recurrence: gates [16, 1024] via matmuls with the weight
  matrices as the MOVING operand; the input projection + bias is batched
  3 steps per stationary (out rows at 32-aligned partition slots), so a
  step costs ~6 chain matmuls + 2 tiny PE transposes of h (back to
  h-major lhsT layout) + 7 elementwise ops — ~23 engine instructions
  versus 39+ for the gate-major formulation.
- Chunk-parallel CRF: 8 time-chunk operators (17x17 per sequence, exp
  space, renormalized every 8 steps) evolve together, each local step
  being 5 wide matmuls + 1 broadcast-multiply; the final 8 tiny matvecs
  per sequence run on host from the downloaded operators. Serial depth
  64 instead of 511.
- Post-schedule passes: duplicate-Ldweights removal, transitive sem-wait
  pruning, and same-engine-order wait dropping kill ~2k Drain/Ldweights
  instructions that the stock Tile lowering emits.
"""

import numpy as np
import ml_dtypes

import bass_rust
import concourse.bass as bass
import concourse.mybir as mybir
import concourse.tile as tile
from concourse.bass_utils import run_bass_kernel_spmd
from concourse.vector_clock import ScopedClock


def _split_drain_and_barrier(self, tick_clock, wait_clock):
    """TileContext tail-drain patch: the walrus in this container rejects >1
    sync wait on a Drain (CoreV3 CTRL_NO_STRUCT), so split the final
    global-clock waits across one Drain per semaphore."""
    nc = self.nc
    drain_inst = nc.sync.drain()
    wait_clock.add_sem_waits(
        drain_inst.ins, ScopedClock({None: tick_clock.global_clock}))
    si = drain_inst.ins.sync_info
    if si is not None and si.on_wait and len(si.on_wait) > 1:
        waits = list(si.on_wait)
        drain_inst.ins.sync_info = bass_rust.SyncInfo(
            on_wait=[waits[0]], on_update=list(si.on_update))
        for w in waits[1:]:
            extra = nc.sync.drain()
            extra.ins.sync_info = bass_rust.SyncInfo(on_wait=[w], on_update=[])
    nc.all_engine_barrier()
    assert self.sems is not None
    popped = nc._tile_sem_poison_stack.pop()
    assert popped is self._sem_poison
    nc.clear_and_free_semaphores(list(self.sems.allocated().values()))
    nc.all_engine_barrier()


tile.TileContext._drain_and_barrier = _split_drain_and_barrier

_orig_lower_ordered_insts = tile.TileContext._lower_ordered_insts


def _dedup_ldweights(postordered_blocks):
    """Remove InstLdweights whose stationary AP is identical to the one
    already loaded (PE keeps the stationary across matmuls with
    ldweights=False). Waits/updates of a removed load are merged into the
    next PE instruction."""
    removed = 0
    for bb_name, insts in postordered_blocks.items():
        last_key = None
        pend_wait, pend_upd = [], []
        out = []
        for inst in insts:
            if isinstance(inst, mybir.InstLdweights):
                key = str(inst.ins[0])
                if key == last_key:
                    si = inst.sync_info
                    if si is not None:
                        pend_wait.extend(si.on_wait or [])
                        pend_upd.extend(si.on_update or [])
                    removed += 1
                    continue
                last_key = key
            elif (pend_wait or pend_upd) and \
                    getattr(inst, "engine", None) == mybir.EngineType.PE:
                si = inst.sync_info
                ow = list(si.on_wait) if si is not None else []
                ou = list(si.on_update) if si is not None else []
                inst.sync_info = bass_rust.SyncInfo(
                    on_wait=pend_wait + ow, on_update=pend_upd + ou)
                pend_wait, pend_upd = [], []
            out.append(inst)
        assert not pend_wait and not pend_upd
        insts[:] = out
    return removed


def _merge_same_sem_waits(waits):
    """Collapse sem-ge waits on the same semaphore to the max value."""
    merged = {}
    out = []
    for w in waits:
        if getattr(w, "wait_mode", None) == "sem-ge-imm" and \
                getattr(w, "wait_reg", None) is None:
            key = (w.sync_type, w.id)
            if key in merged:
                prev = merged[key]
                if w.wait_value > prev.wait_value:
                    out[out.index(prev)] = w
                    merged[key] = w
                continue
            merged[key] = w
        out.append(w)
    return out


# engines with strictly serial in-order execution (full drain between
# ops). Pool (GPSIMD) is excluded: its 8 Q7 cores run concurrently. SP is
# excluded: its dma_starts complete asynchronously, so SP program order
# does not imply its semaphore values.
_SELF_WAIT_ENGINES = (mybir.EngineType.DVE, mybir.EngineType.Activation)


def _transitive_prune_waits(postordered_blocks):
    """Prune waits implied transitively through other waits: if inst X
    waits on sem A at value a, and A's producer itself waited on sem B >=
    b, then an X-wait on B >= b' with b' <= b is redundant. Tracks, per
    single-engine-owned semaphore value, a snapshot of the guaranteed
    vector clock at the producing instruction."""
    from collections import defaultdict
    pruned = 0
    for bb_name, insts in postordered_blocks.items():
        upd_engines = defaultdict(set)
        for inst in insts:
            si = getattr(inst, "sync_info", None)
            if si is not None:
                for u in (si.on_update or []):
                    upd_engines[u.id].add(getattr(inst, "engine", None))
        tracked = {sid for sid, es in upd_engines.items() if len(es) == 1}
        cur = defaultdict(int)        # sem -> current cumulative value
        snap = defaultdict(dict)      # sem -> value -> know dict
        know = defaultdict(dict)      # engine -> sem -> guaranteed value
        for inst in insts:
            si = getattr(inst, "sync_info", None)
            eng = getattr(inst, "engine", None)
            ke = know[eng]
            if si is not None and si.on_wait:
                waits = list(si.on_wait)
                snaps = []
                for w in waits:
                    if (w.sync_type == "semaphore" and w.id in tracked
                            and w.wait_mode == "sem-ge-imm"
                            and getattr(w, "wait_reg", None) is None):
                        snaps.append(snap[w.id].get(w.wait_value, {w.id: w.wait_value}))
                    else:
                        snaps.append(None)
                if len(waits) > 1:
                    kept = []
                    for i, w in enumerate(waits):
                        if (w.sync_type == "semaphore"
                                and w.wait_mode == "sem-ge-imm"
                                and getattr(w, "wait_reg", None) is None):
                            implied = ke.get(w.id, 0) >= w.wait_value
                            if not implied:
                                for j, s in enumerate(snaps):
                                    if j != i and s is not None and \
                                            s.get(w.id, 0) >= w.wait_value:
                                        implied = True
                                        break
                            if implied:
                                pruned += 1
                                continue
                        kept.append(w)
                    if len(kept) != len(waits) and kept:
                        inst.sync_info = bass_rust.SyncInfo(
                            on_wait=kept, on_update=list(si.on_update))
                # merge all waits' knowledge (conditions hold either way)
                for w, s in zip(waits, snaps):
                    if s is not None:
                        for sid, v in s.items():
                            if ke.get(sid, 0) < v:
                                ke[sid] = v
                    if w.sync_type == "semaphore" and \
                            w.wait_mode == "sem-ge-imm" and \
                            getattr(w, "wait_reg", None) is None:
                        if ke.get(w.id, 0) < w.wait_value:
                            ke[w.id] = w.wait_value
            si = getattr(inst, "sync_info", None)
            if si is not None:
                for u in (si.on_update or []):
                    if u.id in tracked and u.update_mode == "sem-inc" \
                            and getattr(u, "update_reg", None) is None:
                        cur[u.id] += u.update_value or 0
                        s = dict(ke)
                        s[u.id] = max(s.get(u.id, 0), cur[u.id])
                        snap[u.id][cur[u.id]] = s
                        if DROP_SELF_WAITS and \
                                ke.get(u.id, 0) < cur[u.id] and \
                                eng in _SELF_WAIT_ENGINES and \
                                upd_engines[u.id] == {eng}:
                            ke[u.id] = cur[u.id]
    return pruned

# CoreSim's race detector does not model same-engine serial ordering, so
# sim-based checkers disable this pass; hardware runs keep it on.
DROP_SELF_WAITS = True


def _drop_self_engine_waits(postordered_blocks):
    """Drop waits that are implied by same-engine program order: DVE/ACT/
    SP/Pool execute serially (full drain between ops), so a wait on the
    engine's own monotonic semaphore for a value already reached by
    preceding same-engine increments in this block is a no-op."""
    from collections import defaultdict
    dropped = 0
    for bb_name, insts in postordered_blocks.items():
        upd_engines = defaultdict(set)
        for inst in insts:
            si = getattr(inst, "sync_info", None)
            if si is not None:
                for u in (si.on_update or []):
                    upd_engines[u.id].add(getattr(inst, "engine", None))
        safe = {sid for sid, es in upd_engines.items()
                if len(es) == 1 and next(iter(es)) in _SELF_WAIT_ENGINES}
        cum = defaultdict(int)
        for inst in insts:
            si = getattr(inst, "sync_info", None)
            eng = getattr(inst, "engine", None)
            if si is not None and si.on_wait and len(si.on_wait) > 1:
                kept = []
                for w in si.on_wait:
                    if (w.sync_type == "semaphore" and w.id in safe
                            and next(iter(upd_engines[w.id])) == eng
                            and w.wait_mode == "sem-ge-imm"
                            and getattr(w, "wait_reg", None) is None
                            and cum[w.id] >= w.wait_value):
                        dropped += 1
                        continue
                    kept.append(w)
                if len(kept) != len(si.on_wait):
                    inst.sync_info = bass_rust.SyncInfo(
                        on_wait=kept, on_update=list(si.on_update))
            si = getattr(inst, "sync_info", None)
            if si is not None:
                for u in (si.on_update or []):
                    if u.id in safe and u.update_mode == "sem-inc" \
                            and getattr(u, "update_reg", None) is None:
                        cum[u.id] += u.update_value or 0
    return dropped


def _split_multi_waits(self, postordered_blocks):
    """Same walrus limitation for scheduled instructions: move excess sync
    waits onto same-engine Drain instructions inserted just before."""
    _dedup_ldweights(postordered_blocks)
    _transitive_prune_waits(postordered_blocks)
    if DROP_SELF_WAITS:
        _drop_self_engine_waits(postordered_blocks)
    for bb_name, insts in postordered_blocks.items():
        out = []
        for inst in insts:
            si = getattr(inst, "sync_info", None)
            if si is not None and si.on_wait and len(si.on_wait) > 1:
                merged = _merge_same_sem_waits(list(si.on_wait))
                if len(merged) != len(si.on_wait):
                    inst.sync_info = bass_rust.SyncInfo(
                        on_wait=merged, on_update=list(si.on_update))
                    si = inst.sync_info
            if si is not None and si.on_wait and len(si.on_wait) > 1:
                waits = list(si.on_wait)
                for k, w in enumerate(waits[1:]):
                    d = mybir.InstDrain(
                        name=f"{inst.name}_ws{k}", engine=inst.engine,
                        ins=[], outs=[],
                        sync_info=bass_rust.SyncInfo(on_wait=[w],
                                                     on_update=[]))
                    out.append(d)
                inst.sync_info = bass_rust.SyncInfo(
                    on_wait=[waits[0]], on_update=list(si.on_update))
            out.append(inst)
        insts[:] = out
    return _orig_lower_ordered_insts(self, postordered_blocks)


tile.TileContext._lower_ordered_insts = _split_multi_waits

F32 = mybir.dt.float32
BF16 = mybir.dt.bfloat16
I32 = mybir.dt.int32
AF = mybir.ActivationFunctionType
ALU = mybir.AluOpType

VOCAB, EMB, HID, NLAB = 20000, 256, 512, 17
H = HID // 2          # 256 per direction
GATES = 4 * H         # 1024
B_FULL, S_FULL = 64, 512
KH = H // 128         # 2
KE = EMB // 128       # 2
BANK = 512            # fp32 elems per PSUM bank


def build_nc(S=S_FULL, BC=16, CHUNK=64, RENORM=8, n_cores=8,
             use_collective=True, phases=4):
    """Build the SPMD Bass program (identical on all cores)."""
    assert S % CHUNK == 0
    NCH = S // CHUNK              # emission chunks
    TOK = S * BC                  # tokens per core
    TPC = CHUNK * BC              # tokens per emission chunk
    assert TPC <= 2 * BANK

    nc = bass.Bass("TRN2", target_bir_lowering=False, num_devices=n_cores)

    # ---------------- DRAM I/O ----------------
    xT_d = nc.dram_tensor("xT", [EMB, TOK], mybir.dt.float8e3,
                          kind="ExternalInput")
    wstat_d = nc.dram_tensor("w_stat", [H, GATES], BF16, kind="ExternalInput")
    win_d = nc.dram_tensor("w_in", [EMB, GATES], BF16, kind="ExternalInput")
    brow_d = nc.dram_tensor("bias_row", [1, GATES], BF16, kind="ExternalInput")
    id_d = nc.dram_tensor("ident", [BC, BC], BF16, kind="ExternalInput")
    wo_d = nc.dram_tensor("wo_stat", [H, NLAB], BF16, kind="ExternalInput")
    bo_d = nc.dram_tensor("bo_row", [1, NLAB], BF16, kind="ExternalInput")
    expT_d = nc.dram_tensor("expT", [NLAB, NLAB], F32, kind="ExternalInput")
    expS_d = nc.dram_tensor("expStart", [NLAB, 1], F32, kind="ExternalInput")
    oh_d = nc.dram_tensor("onehot", [NLAB, TOK], BF16, kind="ExternalInput")
    CCH = 8                       # CRF time chunks (parallel operators)
    LCH = S // CCH                # local steps per chunk
    CB = CCH * BC                 # operator blocks
    W = CB * NLAB                 # state columns
    crfinit_d = nc.dram_tensor("crf_init", [NLAB, W], mybir.dt.float8e3,
                               kind="ExternalInput")
    out_emit_d = nc.dram_tensor("out_emit", [NLAB, BC], F32,
                                kind="ExternalOutput")
    out_R_d = nc.dram_tensor("out_R", [NLAB, W], F32,
                             kind="ExternalOutput")
    out_logs_d = nc.dram_tensor("out_logs", [1, CB], F32,
                                kind="ExternalOutput")
    if use_collective:
        cc_in_d = nc.dram_tensor("cc_in", [NLAB, TOK], BF16, kind="Internal")
        cc_out_d = nc.dram_tensor("cc_out", [2, NLAB, TOK], BF16,
                                  kind="Internal")
    else:
        emf_in_d = nc.dram_tensor("dbg_em_f", [NLAB, TOK], BF16,
                                  kind="ExternalInput")
        emb_in_d = nc.dram_tensor("dbg_em_b", [NLAB, TOK], BF16,
                                  kind="ExternalInput")
        dbg_out_d = nc.dram_tensor("dbg_em_out", [NLAB, TOK], BF16,
                                   kind="ExternalOutput")

    groups = [[i, i + n_cores // 2] for i in range(n_cores // 2)]

    with tile.TileContext(nc) as tc:
        with tc.tile_pool(name="consts", bufs=1) as consts, \
             tc.tile_pool(name="state", bufs=1) as state:
            # ---- constants needed beyond phase 1 ----
            ones_row = consts.tile([1, BANK], BF16, tag="ones_row")
            nc.vector.memset(ones_row[:], 1.0)
            wo_sb = consts.tile([128, KH * NLAB], BF16, tag="wo")
            for k in range(KH):
                nc.sync.dma_start(wo_sb[:, k * NLAB:(k + 1) * NLAB],
                                  wo_d[128 * k:128 * (k + 1), :])
            bo_sb = consts.tile([1, NLAB], BF16, tag="bo")
            nc.sync.dma_start(bo_sb[:], bo_d[:])

            # ---- LSTM state ----
            hs_all = state.tile([128, S + 1, KH, BC], BF16, tag="hs")
            nc.vector.memset(hs_all[:, 0], 0.0)
            c_st = state.tile([BC, H], F32, tag="c")
            nc.vector.memset(c_st[:], 0.0)

            # =============== phase 1: LSTM recurrence (batch-major) ======
            with tc.tile_pool(name="lstmc", bufs=1) as lstmc, \
                 tc.tile_pool(name="gpsum", bufs=1, space="PSUM") as psum, \
                 tc.tile_pool(name="empsum", bufs=1, space="PSUM") as em_ps, \
                 tc.tile_pool(name="step", bufs=3) as step_pool:
                # LSTM-only constants; pool closes after phase 1 to free
                # SBUF for the emission tiles.
                wk = []
                for k in range(KH):
                    t = lstmc.tile([128, GATES], BF16, name=f"wk{k}",
                                   tag=f"wk{k}")
                    nc.sync.dma_start(t[:], wstat_d[128 * k:128 * (k + 1), :])
                    wk.append(t)
                wi = []
                for k in range(KE):
                    t = lstmc.tile([128, GATES], BF16, name=f"wi{k}",
                                   tag=f"wi{k}")
                    nc.sync.dma_start(t[:], win_d[128 * k:128 * (k + 1), :])
                    wi.append(t)
                brow = lstmc.tile([1, GATES], BF16, tag="brow")
                nc.sync.dma_start(brow[:], brow_d[:])
                ident = lstmc.tile([BC, BC], BF16, tag="ident")
                nc.sync.dma_start(ident[:], id_d[:])
                # xT arrives packed fp8 [EMB, S*BC]; upcast to bf16 and
                # spread to 32-col step slots (zero padding between
                # batches) so batched 3-step stationaries hit 32-aligned
                # row slots.
                em_my = lstmc.tile([NLAB, TOK], BF16, tag="em_my")
                xT = []
                for k in range(KE):
                    t8 = lstmc.tile([128, TOK], mybir.dt.float8e3,
                                    name=f"xT8_{k}", tag=f"xT8_{k}")
                    nc.sync.dma_start(t8[:], xT_d[128 * k:128 * (k + 1), :])
                    t = lstmc.tile([128, S * 32], BF16, name=f"xT{k}",
                                   tag=f"xT{k}")
                    nc.vector.memset(t[:], 0.0)
                    nc.vector.tensor_copy(
                        t[:].rearrange("p (s v) -> p s v", s=S,
                                       v=32)[:, :, 0:BC],
                        t8[:].rearrange("p (s b) -> p s b", s=S, b=BC))
                    xT.append(t)
                gp = psum.tile([128, 2, GATES], F32, tag="gp")
                tp = psum.tile([128, KH * BC], F32, tag="tp")

                HCK = CHUNK // 2

                def emit_chunk(ch):
                    """emission GEMMs for hs steps [ch*CHUNK, +CHUNK);
                    interleaved into the step loop so the tail does not
                    serialize after the recurrence (its hs waits are the
                    same ones the next step's matmuls already make)."""
                    ep = em_ps.tile([NLAB, TPC], F32, tag="ep")
                    for hf in range(2):
                        es = ep[:, hf * BANK:(hf + 1) * BANK]
                        nc.tensor.matmul(es, bo_sb[:], ones_row[:, :BANK],
                                         start=True, stop=False)
                        a = ch * CHUNK + hf * HCK + 1
                        for kc in range(KH):
                            rhs = hs_all[:, a:a + HCK, kc, :]
                            nc.tensor.matmul(
                                es, wo_sb[:, kc * NLAB:(kc + 1) * NLAB],
                                rhs, start=False, stop=(kc == KH - 1))
                    nc.scalar.copy(em_my[:, ch * TPC:(ch + 1) * TPC],
                                   ep[:])

                HB = GATES // 2   # psum-bank-sized half (512 f32)
                GRP = 3           # steps per psum group (32-row slots;
                                  # hw limits AP base partition to 0/32/64)
                em_next = 0
                starts = list(range(0, S, GRP))
                for g, s0 in enumerate(starts):
                    n = min(GRP, S - s0)
                    par = g % 2
                    # off-chain: batched input projection + bias for the
                    # 4 steps of this group. xT is host-padded to 32 cols
                    # per step so the stationary [128,128] lands each
                    # step's 16 batch rows at a 32-aligned partition slot.
                    for k in range(KE):
                        xs = xT[k][:, s0 * 32:(s0 + n) * 32]
                        for hf in range(2):
                            nc.tensor.matmul(
                                gp[0:32 * n, par, hf * HB:(hf + 1) * HB],
                                xs, wi[k][:, hf * HB:(hf + 1) * HB],
                                start=(k == 0), stop=False,
                                skip_group_check=True)
                    for hf in range(2):
                        nc.tensor.matmul(
                            gp[0:32 * n, par, hf * HB:(hf + 1) * HB],
                            ones_row[:, :32 * n],
                            brow[:, hf * HB:(hf + 1) * HB],
                            start=False, stop=False, skip_group_check=True)
                    for m in range(n):
                        s = s0 + m
                        ro = 32 * m
                        # chain: recurrent part into this step's row slot
                        for k in range(KH):
                            for hf in range(2):
                                nc.tensor.matmul(
                                    gp[ro:ro + BC, par,
                                       hf * HB:(hf + 1) * HB],
                                    hs_all[:, s, k, :],
                                    wk[k][:, hf * HB:(hf + 1) * HB],
                                    start=False, stop=(k == KH - 1),
                                    skip_group_check=True)
                        # elementwise, batch-major [16, *]; gate order
                        # i,f,o,g; reads shift partitions 32m -> 0
                        T = step_pool.tile([BC, GATES], F32, tag="T")
                        nc.scalar.activation(T[:, 0:3 * H],
                                             gp[ro:ro + BC, par, 0:3 * H],
                                             AF.Sigmoid)
                        nc.scalar.activation(T[:, 3 * H:],
                                             gp[ro:ro + BC, par, 3 * H:],
                                             AF.Tanh)
                        Q = step_pool.tile([BC, H], F32, tag="Q")
                        R = step_pool.tile([BC, H], F32, tag="R")
                        # R first: it only needs the sigmoid pass, so the
                        # in-order DVE can start it while tanh(g) runs
                        nc.vector.tensor_tensor(R[:], T[:, H:2 * H],
                                                c_st[:], op=ALU.mult)
                        nc.vector.tensor_tensor(Q[:], T[:, 0:H],
                                                T[:, 3 * H:], op=ALU.mult)
                        nc.vector.tensor_tensor(c_st[:], Q[:], R[:],
                                                op=ALU.add)
                        tc_t = step_pool.tile([BC, H], F32, tag="tc")
                        nc.scalar.activation(tc_t[:], c_st[:], AF.Tanh)
                        hbm = step_pool.tile([BC, H], BF16, tag="hbm")
                        nc.vector.tensor_tensor(hbm[:], T[:, 2 * H:3 * H],
                                                tc_t[:], op=ALU.mult)
                        # transpose h back to h-major for the next lhsT
                        for k in range(KH):
                            nc.tensor.matmul(tp[:, k * BC:(k + 1) * BC],
                                             hbm[:, k * 128:(k + 1) * 128],
                                             ident[:], start=True,
                                             stop=True)
                        nc.scalar.copy(hs_all[:, s + 1], tp[:])
                    while (em_next + 1) * CHUNK <= s0 + n:
                        emit_chunk(em_next)
                        em_next += 1
                assert em_next == NCH
                if use_collective:
                    nc.sync.dma_start(cc_in_d[:], em_my[:])
                else:
                    nc.sync.dma_start(dbg_out_d[:], em_my[:])

            # =============== phase 2: partial emissions ===============
            if phases < 2:
                return nc
            with tc.tile_pool(name="emis", bufs=1) as emis:
                # =============== phase 3: exchange + CRF inputs ========
                if phases < 3:
                    return nc
                if use_collective:
                    nc.gpsimd.collective_compute(
                        "AllGather", ALU.bypass, replica_groups=groups,
                        ins=[cc_in_d[:]], outs=[cc_out_d[:]])
                em_f = emis.tile([NLAB, TOK], BF16, tag="em_f")
                em_b = emis.tile([NLAB, TOK], BF16, tag="em_b")
                ems = emis.tile([NLAB, TOK], F32, tag="ems")
                if use_collective:
                    nc.sync.dma_start(em_f[:], cc_out_d[0])
                    nc.sync.dma_start(em_b[:], cc_out_d[1])
                else:
                    nc.sync.dma_start(em_f[:], emf_in_d[:])
                    nc.sync.dma_start(em_b[:], emb_in_d[:])
                em_b_rev = em_b[:].rearrange("p (s b) -> p s b",
                                             s=S, b=BC)[:, ::-1, :]
                nc.vector.tensor_tensor(ems[:], em_f[:], em_b_rev,
                                        op=ALU.add)
                eem = emis.tile([NLAB, TOK], F32, tag="eem")
                nc.scalar.activation(eem[:], ems[:], AF.Exp)

                # gold-label emission sums
                oh_sb = emis.tile([NLAB, TOK], BF16, tag="oh")
                nc.sync.dma_start(oh_sb[:], oh_d[:])
                # ems is dead after the exp; reuse it for the gold product
                nc.vector.tensor_tensor(ems[:], ems[:], oh_sb[:],
                                        op=ALU.mult)
                emit_bt = emis.tile([NLAB, BC], F32, tag="emit_bt")
                nc.vector.tensor_reduce(
                    emit_bt[:],
                    ems[:].rearrange("p (s b) -> p b s", s=S, b=BC),
                    axis=mybir.AxisListType.X, op=ALU.add)
                nc.sync.dma_start(out_emit_d[:], emit_bt[:])

                # =============== phase 4: CRF forward scan =============
                if phases < 4:
                    return nc
                # Chunk-parallel CRF: evolve 8 time-chunk operators (17x17
                # per sequence, exp space) simultaneously; each local step
                # is 5 wide matmuls + 1 broadcast-multiply instead of a
                # 511-long matmul->multiply serial chain. The tiny final
                # operator chain (8 matvecs per sequence) runs on host.
                with tc.tile_pool(name="crfc", bufs=1) as crf_c, \
                     tc.tile_pool(name="crfp", bufs=2) as crf_p, \
                     tc.tile_pool(name="crfps", bufs=1,
                                  space="PSUM") as crf_ps:
                    expT_sb = crf_c.tile([NLAB, NLAB], F32, tag="expT")
                    nc.sync.dma_start(expT_sb[:], expT_d[:])
                    expS_sb = crf_c.tile([NLAB, 1], F32, tag="expS")
                    nc.sync.dma_start(expS_sb[:], expS_d[:])
                    ones17 = crf_c.tile([NLAB, 1], F32, tag="ones17")
                    nc.vector.memset(ones17[:], 1.0)
                    ones117 = crf_c.tile([1, NLAB], F32, tag="ones117")
                    nc.vector.memset(ones117[:], 1.0)
                    logs = crf_c.tile([1, CB], F32, tag="logs")
                    nc.vector.memset(logs[:], 0.0)
                    Rst = crf_c.tile([NLAB, W], F32, tag="Rst")
                    ini8 = crf_c.tile([NLAB, W], mybir.dt.float8e3,
                                      tag="ini8")
                    nc.sync.dma_start(ini8[:], crfinit_d[:])
                    nc.vector.tensor_copy(Rst[:], ini8[:])
                    qt = crf_ps.tile([NLAB, W], F32, tag="qt")

                    eemv = eem[:].rearrange("p (c t b) -> p c t b",
                                            c=CCH, t=LCH, b=BC)
                    Rv = Rst[:].rearrange("p (c b j) -> p c b j",
                                          c=CCH, b=BC, j=NLAB)

                    def regions(lo, hi):
                        edges = [lo] + [e for e in range(BANK, hi, BANK)
                                        if e > lo] + [hi]
                        return zip(edges[:-1], edges[1:])

                    BW = BC * NLAB   # columns per chunk block (272)
                    for t in range(LCH):
                        lo = BW if t == 0 else 0
                        for (ra, rb) in regions(lo, W):
                            nc.tensor.matmul(qt[:, ra:rb], expT_sb[:],
                                             Rst[:, ra:rb],
                                             start=True, stop=True)
                        if t == 0:
                            # chunk 0 starts as the plain alpha vector
                            nc.vector.tensor_scalar_mul(
                                Rv[:, 0, :, 0], eem[:, 0:BC], expS_sb[:])
                            em_bc = eemv[:, 1:, t, :].broadcast_to(
                                (NLAB, CCH - 1, BC, NLAB))
                            nc.vector.tensor_tensor(
                                Rv[:, 1:], qt[:, BW:].rearrange(
                                    "p (c b j) -> p c b j", c=CCH - 1,
                                    b=BC, j=NLAB),
                                em_bc, op=ALU.mult)
                        else:
                            em_bc = eemv[:, :, t, :].broadcast_to(
                                (NLAB, CCH, BC, NLAB))
                            nc.vector.tensor_tensor(
                                Rv[:], qt[:].rearrange(
                                    "p (c b j) -> p c b j", c=CCH,
                                    b=BC, j=NLAB),
                                em_bc, op=ALU.mult)
                        if t % RENORM == RENORM - 1:
                            Rj = crf_p.tile([NLAB, CB], F32, tag="Rj")
                            nc.vector.tensor_reduce(
                                Rj[:], Rv[:],
                                axis=mybir.AxisListType.X, op=ALU.add)
                            cs = crf_ps.tile([1, CB], F32, tag="cs")
                            nc.tensor.matmul(cs[:], ones17[:], Rj[:],
                                             start=True, stop=True)
                            sinv = crf_p.tile([1, CB], F32, tag="sinv")
                            nc.vector.reciprocal(sinv[:], cs[:])
                            bc17 = crf_ps.tile([NLAB, CB], F32,
                                               tag="bc17")
                            nc.tensor.matmul(bc17[:], ones117[:],
                                             sinv[:], start=True,
                                             stop=True)
                            sc_bc = bc17[:].rearrange(
                                "p (c b) -> p c b", c=CCH,
                                b=BC).broadcast_to(
                                (NLAB, CCH, BC, NLAB))
                            nc.vector.tensor_tensor(Rv[:], Rv[:], sc_bc,
                                                    op=ALU.mult)
                            lg = crf_p.tile([1, CB], F32, tag="lg")
                            nc.scalar.activation(lg[:], cs[:], AF.Ln)
                            nc.vector.tensor_tensor(logs[:], logs[:],
                                                    lg[:], op=ALU.add)
                    nc.sync.dma_start(out_R_d[:], Rst[:])
                    nc.sync.dma_start(out_logs_d[:], logs[:])

    return nc


# ====================== host side ======================

def _perm_gates(w, order=(0, 1, 3, 2)):
    """reorder gate blocks [i,f,g,o] -> [i,f,o,g] along axis 0"""
    blocks = np.split(np.asarray(w), 4, axis=0)
    return np.concatenate([blocks[i] for i in order], axis=0)


def _bf(x):
    return np.ascontiguousarray(
        np.asarray(x, dtype=np.float32)).astype(ml_dtypes.bfloat16)


def make_in_maps(inputs, S=S_FULL, BC=16, n_cores=8, use_collective=True,
                 dbg_em=None):
    chars = np.asarray(inputs["chars"], dtype=np.int64)
    labels = np.asarray(inputs["labels"], dtype=np.int64)
    npair = n_cores // 2
    emb_f = np.asarray(inputs["emb"], np.float32)
    ident = np.eye(BC, dtype=np.float32)
    # one embedding gather + transpose for all cores; bwd cores reuse the
    # fwd gather through a reversed view (fp8 cast per core below)
    x_all = emb_f[chars[:, :S]]                         # [B, S, EMB]
    xT_all = np.ascontiguousarray(
        x_all.transpose(2, 1, 0))                       # [EMB, S, B]

    in_maps = []
    for core in range(n_cores):
        is_bwd = core >= npair
        q = core % npair
        ch_q = chars[q * BC:(q + 1) * BC, :S]          # [BC, S]
        lb_q = labels[q * BC:(q + 1) * BC, :S]
        d = "b" if is_bwd else "f"
        w_ih = _perm_gates(inputs[f"w_ih_{d}"])
        w_hh = _perm_gates(inputs[f"w_hh_{d}"])
        bias = _perm_gates(np.asarray(inputs[f"b_ih_{d}"]) +
                           np.asarray(inputs[f"b_hh_{d}"]))
        # xT [EMB, S*BC], token col = s*BC + b (bwd: reversed step order)
        xTv = xT_all[:, :, q * BC:(q + 1) * BC]
        if is_bwd:
            xTv = xTv[:, ::-1, :]
        xT = xTv.reshape(EMB, S * BC)
        w_out = np.asarray(inputs["w_out"], np.float32)
        wo_half = w_out[:, H:] if is_bwd else w_out[:, :H]
        bo = np.zeros(NLAB, np.float32) if is_bwd \
            else np.asarray(inputs["b_out"], np.float32)
        onehot = (lb_q.T.reshape(1, -1) ==
                  np.arange(NLAB)[:, None]).astype(np.float32)
        m = {
            "xT": xT.astype(ml_dtypes.float8_e3m4),
            "w_stat": _bf(w_hh.T),
            "w_in": _bf(w_ih.T),
            "bias_row": _bf(bias.reshape(1, -1)),
            "ident": ident.astype(ml_dtypes.bfloat16),
            "wo_stat": _bf(wo_half.T),
            "bo_row": _bf(bo.reshape(1, -1)),
            "expT": np.ascontiguousarray(
                np.exp(np.asarray(inputs["trans"], np.float32))),
            "expStart": np.exp(np.asarray(
                inputs["start_trans"], np.float32)).reshape(-1, 1),
            "crf_init": _crf_init(S, BC),
            "onehot": _bf(onehot),
        }
        if not use_collective:
            m["dbg_em_f"] = _bf(dbg_em[q][0])
            m["dbg_em_b"] = _bf(dbg_em[q][1])
        in_maps.append(m)
    return in_maps


def _crf_init(S, BC, CCH=8):
    """Initial chunk operators: identity blocks for chunks 1.., zeros for
    chunk 0 (which starts from the alpha vector on device)."""
    init = np.zeros((NLAB, CCH, BC, NLAB), np.float32)
    init[:, 1:, :, :] = np.eye(NLAB, dtype=np.float32)[:, None, None, :]
    return np.ascontiguousarray(
        init.reshape(NLAB, CCH * BC * NLAB)).astype(ml_dtypes.float8_e3m4)


def combine_logz(r, end_trans, S=S_FULL, BC=16, CCH=8):
    """Host tail of the chunk-parallel CRF: chain the 8 chunk operators
    per sequence and apply end transitions. Returns [BC] logZ values."""
    R = np.asarray(r["out_R"], np.float64).reshape(NLAB, CCH, BC, NLAB)
    logs = np.asarray(r["out_logs"], np.float64).reshape(CCH, BC)
    eT = np.exp(np.asarray(end_trans, np.float64))
    out = np.empty(BC)
    for b in range(BC):
        P = R[:, 0, b, 0]
        for c in range(1, CCH):
            P = R[:, c, b, :] @ P
        out[b] = np.log(eT @ P) + logs[:, b].sum()
    return out


def static_score(inputs, S=S_FULL):
    """label-only part of the numerator (host, from inputs only)"""
    labels = np.asarray(inputs["labels"], dtype=np.int64)[:, :S]
    st = np.asarray(inputs["start_trans"], np.float64)
    et = np.asarray(inputs["end_trans"], np.float64)
    tr = np.asarray(inputs["trans"], np.float64)
    sc = st[labels[:, 0]] + et[labels[:, -1]]
    sc = sc + tr[labels[:, :-1], labels[:, 1:]].sum(axis=1)
    return float(sc.sum())


def reduce_outputs(results, inputs, n_cores=8, S=S_FULL):
    total = 0.0
    for q in range(n_cores // 2):
        r = results[q]
        total += float(combine_logz(r, inputs["end_trans"], S=S).sum())
        total -= float(np.asarray(r["out_emit"], np.float64).sum())
    total -= static_score(inputs, S=S)
    return np.float32(total)


def kernel(**inputs) -> np.ndarray:
    S, BC, n_cores = S_FULL, 16, 8
    nc = build_nc(S=S, BC=BC, n_cores=n_cores)
    in_maps = make_in_maps(inputs, S=S, BC=BC, n_cores=n_cores)
    res = run_bass_kernel_spmd(nc, in_maps, core_ids=list(range(n_cores)))
    return reduce_outputs(res.results, inputs, n_cores=n_cores, S=S)
